# revision 18
# baseline (speedup 1.0000x reference)
"""Trainium2 Bass kernel for nn_PhaseClusterAdapter.

Strategy (8 NeuronCores, data-parallel over batch n=16 -> 2 per core):
  - sim dots d[p,n,s,k] = sum_c x * proto_norm on PE (batched small matmuls,
    c-major natural layout, fp32).
  - hard assignment idx = argmax_k d (scale-invariant, so no x-normalization
    needed on this path).
  - masked max-pool ("clustered") via a fused scalar_tensor_tensor ladder in
    q=(n,p)-partition layout: acc_k = (x_s * m_k,s[q]) max acc_k.  The s=0
    step is a tensor_scalar mult, which reproduces the reference
    max_s(onehot*x) semantics exactly (zeros enter via excluded steps).
  - x row-norms via ACT Square + PE ones-matmul column sums (c-layout),
    softmax / cluster_indices path on small (n,s)-partition tiles.
  - y1 = w1 @ clustered on PE after PE-transposing clustered to kc-major.
  - BatchNorm stats: per-core partial sums + tiny AllReduce (sum, sumsq).
  - gelu fused into one ACT op (scale/bias per-partition), w2 matmul, +b2.

Host side does only sharding/layout marshalling: batch slicing, a transposed
copy of x (q-layout), prototype l2-normalization, w1/w2 transposes.
"""

import numpy as np

import concourse.bass as bass
import concourse.bacc as bacc
import concourse.tile as tile
from concourse import mybir
from concourse.bass_utils import run_bass_kernel_spmd
from concourse.masks import make_identity

F32 = mybir.dt.float32
I32 = mybir.dt.int32
AL = mybir.AluOpType
AF = mybir.ActivationFunctionType

NCORES = 8
N, C, S, H, W = 16, 256, 30, 16, 11
P = H * W            # 176
K = 4
NLOC = N // NCORES   # 2
Q = NLOC * P         # 352
NTOT = N * P         # BN count = 2816
EPS_BN = 1e-5
BIG = 1.0e6

_CACHE = {}


def _build(inv_temp: float, use_collective: bool = True):
    nc = bacc.Bacc("TRN2", target_bir_lowering=False, debug=False,
                   num_devices=NCORES)

    xc_in = nc.dram_tensor("xc", [NLOC, C, S, P], F32, kind="ExternalInput")
    xq_in = nc.dram_tensor("xq", [NLOC, P, S, C], F32, kind="ExternalInput")
    pnc_in = nc.dram_tensor("pnc", [C, P, K], F32, kind="ExternalInput")
    w1t_in = nc.dram_tensor("w1t", [K * C, C], F32, kind="ExternalInput")
    w2t_in = nc.dram_tensor("w2t", [C, C], F32, kind="ExternalInput")
    gam_in = nc.dram_tensor("gam2", [128, 2], F32, kind="ExternalInput")
    bet_in = nc.dram_tensor("bet2", [128, 2], F32, kind="ExternalInput")
    b2_in = nc.dram_tensor("b22", [128, 2], F32, kind="ExternalInput")

    y_out = nc.dram_tensor("y_out", [NLOC, C, P], F32, kind="ExternalOutput")
    ci_out = nc.dram_tensor("ci_out", [NLOC, S], I32, kind="ExternalOutput")

    # q-tiles: A = (nl0, p0:128), B = (nl1, p0:128), C = (nl0,p128:176)+(nl1,p128:176)
    QT = [("A", [(0, 0, 128, 0)]), ("B", [(1, 0, 128, 0)]),
          ("C", [(0, 128, 48, 0), (1, 128, 48, 64)])]

    with tile.TileContext(nc) as tc:
        import contextlib
        ctx = contextlib.ExitStack()
        with ctx:
            perm = ctx.enter_context(tc.tile_pool(name="perm", bufs=1))
            xcp = ctx.enter_context(tc.tile_pool(name="xcp", bufs=2))
            clp = ctx.enter_context(tc.tile_pool(name="clp", bufs=1))
            ps_d = ctx.enter_context(tc.tile_pool(name="ps_d", bufs=1, space="PSUM"))
            ps_t = ctx.enter_context(tc.tile_pool(name="ps_t", bufs=2, space="PSUM"))
            ps_y = ctx.enter_context(tc.tile_pool(name="ps_y", bufs=1, space="PSUM"))
            ps_o = ctx.enter_context(tc.tile_pool(name="ps_o", bufs=1, space="PSUM"))
            ps_s = ctx.enter_context(tc.tile_pool(name="ps_s", bufs=1, space="PSUM"))
            dram = ctx.enter_context(tc.tile_pool(name="dram", bufs=1, space="DRAM"))

            # ---- constants / params ----
            pn_sb = []
            for ch in range(2):
                t = perm.tile([128, P, K], F32, tag=f"pn{ch}")
                nc.sync.dma_start(t[:], pnc_in[ch * 128:(ch + 1) * 128])
                pn_sb.append(t)
            w1sb = perm.tile([128, 8, C], F32, tag="w1sb")
            for a in range(8):
                nc.sync.dma_start(w1sb[:, a, :], w1t_in[a * 128:(a + 1) * 128, :])
            w2sb = perm.tile([128, 2, C], F32, tag="w2sb")
            for a in range(2):
                nc.sync.dma_start(w2sb[:, a, :], w2t_in[a * 128:(a + 1) * 128, :])
            gam = perm.tile([128, 2], F32, tag="gam")
            nc.sync.dma_start(gam[:], gam_in[:])
            bet = perm.tile([128, 2], F32, tag="bet")
            nc.sync.dma_start(bet[:], bet_in[:])
            b2s = perm.tile([128, 2], F32, tag="b2s")
            nc.sync.dma_start(b2s[:], b2_in[:])
            ident = perm.tile([128, 128], F32, tag="ident")
            make_identity(nc, ident)
            iota_i = perm.tile([30, K], I32, tag="iota_i")
            nc.gpsimd.iota(iota_i[:], pattern=[[1, K]], base=0,
                           channel_multiplier=0)
            iotak = perm.tile([30, K], F32, tag="iotak")
            nc.vector.tensor_copy(iotak[:], iota_i[:])

            # ---- xq loads (early, independent) ----
            xq_t = {}
            for name, parts in QT:
                t = clp.tile([128, S, C], F32, tag=f"xq{name}")
                if name == "C":
                    nc.gpsimd.memset(t[32:64, :, :], 0.0)
                    nc.gpsimd.memset(t[96:128, :, :], 0.0)
                for (nl, p0, cnt, qd) in parts:
                    nc.sync.dma_start(t[qd:qd + cnt, :, :],
                                      xq_in[nl, p0:p0 + cnt])
                xq_t[name] = t

            # ---- row norms from xq: ACT Square with accumulate over free ----
            sqj = perm.tile([128, C], F32, tag="sqj")
            rnq = {}
            for name, parts in QT:
                t = perm.tile([128, S], F32, tag=f"rnq{name}")
                for s in range(S):
                    nc.scalar.activation(
                        sqj[:], xq_t[name][:, s, :], AF.Square,
                        accum_out=t[:, s:s + 1])
                rnq[name] = t
            rns = []
            for nl in range(NLOC):
                rn = perm.tile([30, P], F32, tag=f"rns{nl}")
                rns.append(rn)
            for name, parts in QT:
                for (nl, p0, cnt, qd) in parts:
                    pst = ps_s.tile([30, 128], F32, tag="rnt")
                    nc.tensor.transpose(pst[:, 0:cnt],
                                        rnq[name][qd:qd + cnt, :],
                                        ident[qd:qd + cnt, qd:qd + cnt])
                    nc.scalar.copy(rns[nl][:, p0:p0 + cnt], pst[:, 0:cnt])

            # ---- per-nl: x load, sim dots, idx ----
            dsb = []      # [30, P*K] raw dots per nl
            idx_f = []    # [30, P] argmax_k as f32 per nl
            for nl in range(NLOC):
                xct = []
                for ch in range(2):
                    t = xcp.tile([128, S, P], F32, tag="xc")
                    nc.sync.dma_start(
                        t[:], xc_in[nl, ch * 128:(ch + 1) * 128])
                    xct.append(t)

                # sim dots: per (pblk, p): psum [30, 88*4]
                d = perm.tile([30, P, K], F32, tag=f"dsb{nl}")
                for pb in range(2):
                    dp = ps_d.tile([30, 88 * K], F32, tag="dp")
                    for j in range(88):
                        p = pb * 88 + j
                        for ch in range(2):
                            nc.tensor.matmul(
                                dp[:, j * K:(j + 1) * K],
                                xct[ch][:, :, p],
                                pn_sb[ch][:, p, :],
                                start=(ch == 0), stop=(ch == 1))
                    nc.vector.tensor_copy(
                        d[:, pb * 88:(pb + 1) * 88, :],
                        dp[:].rearrange("s (j k) -> s j k", k=K))
                dsb.append(d)


                # idx = argmax_k d  (first-max semantics)
                rmaxd = perm.tile([30, P], F32, tag=f"rmaxd{nl}")
                nc.vector.tensor_reduce(
                    rmaxd[:], d[:], axis=mybir.AxisListType.X, op=AL.max)
                pen = perm.tile([30, P, K], F32, tag="pen")
                nc.vector.tensor_tensor(
                    out=pen[:], in0=d[:],
                    in1=rmaxd[:].unsqueeze(2).broadcast_to((S, P, K)),
                    op=AL.is_lt)
                nc.vector.scalar_tensor_tensor(
                    out=pen[:], in0=pen[:], scalar=BIG,
                    in1=iotak[:].unsqueeze(1).broadcast_to((S, P, K)),
                    op0=AL.mult, op1=AL.add)
                idf = perm.tile([30, P], F32, tag=f"idf{nl}")
                nc.vector.tensor_reduce(
                    idf[:], pen[:], axis=mybir.AxisListType.X, op=AL.min)
                idx_f.append(idf)

            # ---- masks in q-layout + ladder ----
            clq = {}
            for name, parts in QT:
                idxq = perm.tile([128, S], F32, tag=f"idxq{name}")
                if name == "C":
                    nc.vector.memset(idxq[:], 0.0)
                for (nl, p0, cnt, qd) in parts:
                    pst = ps_s.tile([128, S], F32, tag="idxt")
                    nc.tensor.transpose(pst[0:cnt, :],
                                        idx_f[nl][:, p0:p0 + cnt],
                                        ident[0:S, 0:S])
                    nc.scalar.copy(idxq[qd:qd + cnt, :], pst[0:cnt, :])
                msk = perm.tile([128, K, S], F32, tag=f"msk{name}")
                for k in range(K):
                    nc.vector.tensor_scalar(
                        out=msk[:, k, :], in0=idxq[:], scalar1=float(k),
                        scalar2=None, op0=AL.is_equal)
                xq = xq_t[name]
                for k in range(K):
                    acc = clp.tile([128, C], F32, tag=f"cl{name}{k}")
                    nc.vector.tensor_scalar(
                        out=acc[:], in0=xq[:, 0, :], scalar1=msk[:, k, 0:1],
                        scalar2=None, op0=AL.mult)
                    for s in range(1, S):
                        nc.vector.scalar_tensor_tensor(
                            out=acc[:], in0=xq[:, s, :], scalar=msk[:, k, s:s + 1],
                            in1=acc[:], op0=AL.mult, op1=AL.max)
                    clq[(name, k)] = acc

            # ---- transpose clustered to kc-major: clT[(k,ch)] = [128c, 352q] ----
            clT = {}
            qoff = {"A": 0, "B": 128, "C": 256}
            for k in range(K):
                for ch in range(2):
                    t = clp.tile([128, Q], F32, tag=f"clT{k}{ch}")
                    for name, parts in QT:
                        col = qoff[name]
                        for (nl, p0, cnt, qd) in parts:
                            pst = ps_t.tile([128, cnt], F32, tag="pst")
                            nc.tensor.matmul(
                                pst[:],
                                clq[(name, k)][qd:qd + cnt,
                                               ch * 128:(ch + 1) * 128],
                                ident[qd:qd + cnt, qd:qd + cnt],
                                is_transpose=True, start=True, stop=True)
                            nc.scalar.copy(t[:, col:col + cnt], pst[:])
                            col += cnt
                    clT[(k, ch)] = t

            # ---- y1 = w1 @ clustered ----
            y1sb = []
            for oh in range(2):
                psy = ps_y.tile([128, Q], F32, tag="psy")
                for a in range(8):  # kc chunk = (k, ch)
                    k, ch = a // 2, a % 2
                    nc.tensor.matmul(
                        psy[:], w1sb[:, a, oh * 128:(oh + 1) * 128],
                        clT[(k, ch)][:], start=(a == 0), stop=(a == 7))
                t = clp.tile([128, Q], F32, tag=f"y1sb{oh}")
                nc.scalar.copy(t[:], psy[:])
                y1sb.append(t)

            # ---- BN partial stats + AllReduce ----
            stats = perm.tile([128, 4], F32, tag="stats")  # s1_h0 s2_h0 s1_h1 s2_h1
            sqjunk = clp.tile([128, Q], F32, tag="sqjunk")
            for oh in range(2):
                nc.vector.tensor_reduce(
                    stats[:, 2 * oh:2 * oh + 1], y1sb[oh][:],
                    axis=mybir.AxisListType.X, op=AL.add)
                nc.vector.tensor_tensor(
                    out=sqjunk[:], in0=y1sb[oh][:], in1=y1sb[oh][:],
                    op=AL.mult)
                nc.vector.tensor_reduce(
                    stats[:, 2 * oh + 1:2 * oh + 2], sqjunk[:],
                    axis=mybir.AxisListType.X, op=AL.add)
            st_in = dram.tile([128, 4], F32)
            st_out = dram.tile([128, 4], F32)
            nc.sync.dma_start(st_in[:], stats[:])
            if use_collective:
                nc.gpsimd.collective_compute(
                    "AllReduce", AL.add, replica_groups=[list(range(NCORES))],
                    ins=[st_in.opt()], outs=[st_out.opt()])
            else:
                nc.sync.dma_start(st_out[:], st_in[:])
            stat2 = perm.tile([128, 4], F32, tag="stat2")
            nc.sync.dma_start(stat2[:], st_out[:])

            # ---- softmax / cluster_indices path (fills AllReduce latency) ----
            for nl in range(NLOC):
                rn, d = rns[nl], dsb[nl]
                # rsqrt with one NR polish
                rcp = perm.tile([30, P], F32, tag=f"rcp{nl}")
                nc.vector.reciprocal(rcp[:], rn[:])
                rs = perm.tile([30, P], F32, tag=f"rs{nl}")
                nc.scalar.activation(rs[:], rcp[:], AF.Sqrt)
                t1 = perm.tile([30, P], F32, tag=f"t1{nl}")
                nc.vector.tensor_tensor(out=t1[:], in0=rs[:], in1=rs[:], op=AL.mult)
                nc.vector.tensor_tensor(out=t1[:], in0=t1[:], in1=rn[:], op=AL.mult)
                nc.vector.tensor_scalar(out=t1[:], in0=t1[:], scalar1=-0.5,
                                        scalar2=1.5, op0=AL.mult, op1=AL.add)
                nc.vector.tensor_tensor(out=rs[:], in0=rs[:], in1=t1[:], op=AL.mult)
                # sim = d * rnorm ; softmax over k; mean over p
                sim = perm.tile([30, P, K], F32, tag="sim")
                nc.vector.tensor_tensor(
                    out=sim[:], in0=d[:],
                    in1=rs[:].unsqueeze(2).broadcast_to((S, P, K)),
                    op=AL.mult)
                smax = perm.tile([30, P], F32, tag=f"smax{nl}")
                nc.vector.tensor_reduce(
                    smax[:], sim[:], axis=mybir.AxisListType.X, op=AL.max)
                nc.vector.tensor_tensor(
                    out=sim[:], in0=sim[:],
                    in1=smax[:].unsqueeze(2).broadcast_to((S, P, K)),
                    op=AL.subtract)
                nc.scalar.activation(sim[:], sim[:], AF.Exp, scale=inv_temp)
                ssum = perm.tile([30, P], F32, tag=f"ssum{nl}")
                nc.vector.tensor_reduce(
                    ssum[:], sim[:], axis=mybir.AxisListType.X, op=AL.add)
                nc.vector.reciprocal(ssum[:], ssum[:])
                nc.vector.tensor_tensor(
                    out=sim[:], in0=sim[:],
                    in1=ssum[:].unsqueeze(2).broadcast_to((S, P, K)),
                    op=AL.mult)
                avg = perm.tile([30, K], F32, tag=f"avg{nl}")
                nc.vector.tensor_reduce(
                    avg[:], sim[:].rearrange("s p k -> s k p"),
                    axis=mybir.AxisListType.X, op=AL.add)
                amax = perm.tile([30, 1], F32, tag=f"amax{nl}")
                nc.vector.tensor_reduce(
                    amax[:], avg[:], axis=mybir.AxisListType.X, op=AL.max)
                pen = perm.tile([30, K], F32, tag=f"cpen{nl}")
                nc.vector.tensor_tensor(
                    out=pen[:], in0=avg[:],
                    in1=amax[:].broadcast_to((S, K)), op=AL.is_lt)
                nc.vector.scalar_tensor_tensor(
                    out=pen[:], in0=pen[:], scalar=BIG, in1=iotak[:],
                    op0=AL.mult, op1=AL.add)
                cif = perm.tile([30, 1], F32, tag=f"cif{nl}")
                nc.vector.tensor_reduce(
                    cif[:], pen[:], axis=mybir.AxisListType.X, op=AL.min)
                cii = perm.tile([30, 1], I32, tag=f"cii{nl}")
                nc.vector.tensor_copy(cii[:], cif[:])
                nc.sync.dma_start(ci_out[nl, :], cii[:, 0])

            # ---- BN scale/bias from reduced stats ----
            scale = perm.tile([128, 2], F32, tag="scale")
            shift = perm.tile([128, 2], F32, tag="shift")
            mean = perm.tile([128, 2], F32, tag="mean")
            var = perm.tile([128, 2], F32, tag="var")
            for oh in range(2):
                cnt = NTOT if use_collective else Q
                nc.vector.tensor_scalar(
                    out=mean[:, oh:oh + 1], in0=stat2[:, 2 * oh:2 * oh + 1],
                    scalar1=1.0 / cnt, scalar2=None, op0=AL.mult)
                nc.vector.tensor_scalar(
                    out=var[:, oh:oh + 1], in0=stat2[:, 2 * oh + 1:2 * oh + 2],
                    scalar1=1.0 / cnt, scalar2=None, op0=AL.mult)
            nc.vector.tensor_tensor(
                out=scale[:], in0=mean[:], in1=mean[:], op=AL.mult)
            nc.vector.tensor_tensor(
                out=var[:], in0=var[:], in1=scale[:], op=AL.subtract)
            nc.vector.tensor_scalar(
                out=var[:], in0=var[:], scalar1=EPS_BN, scalar2=None, op0=AL.add)
            rstd = perm.tile([128, 2], F32, tag="rstd")
            nc.vector.reciprocal(rstd[:], var[:])
            nc.scalar.activation(rstd[:], rstd[:], AF.Sqrt)
            t2 = perm.tile([128, 2], F32, tag="t2bn")
            nc.vector.tensor_tensor(out=t2[:], in0=rstd[:], in1=rstd[:], op=AL.mult)
            nc.vector.tensor_tensor(out=t2[:], in0=t2[:], in1=var[:], op=AL.mult)
            nc.vector.tensor_scalar(out=t2[:], in0=t2[:], scalar1=-0.5,
                                    scalar2=1.5, op0=AL.mult, op1=AL.add)
            nc.vector.tensor_tensor(out=rstd[:], in0=rstd[:], in1=t2[:], op=AL.mult)
            nc.vector.tensor_tensor(out=scale[:], in0=rstd[:], in1=gam[:], op=AL.mult)
            nc.vector.tensor_tensor(out=shift[:], in0=mean[:], in1=scale[:], op=AL.mult)
            nc.vector.tensor_tensor(out=shift[:], in0=bet[:], in1=shift[:], op=AL.subtract)

            # ---- gelu + w2 + b2 + out ----
            ysb = y1sb
            for oh in range(2):
                nc.scalar.activation(
                    y1sb[oh][:], y1sb[oh][:], AF.Gelu,
                    bias=shift[:, oh:oh + 1], scale=scale[:, oh:oh + 1])
            for oh in range(2):
                pso = ps_o.tile([128, Q], F32, tag="pso")
                for a in range(2):
                    nc.tensor.matmul(
                        pso[:], w2sb[:, a, oh * 128:(oh + 1) * 128],
                        ysb[a][:], start=(a == 0), stop=(a == 1))
                outsb = clp.tile([128, Q], F32, tag=f"outsb{oh}")
                nc.scalar.activation(
                    outsb[:], pso[:], AF.Identity, bias=b2s[:, oh:oh + 1])
                # out cols: A=(nl0,p<128), B=(nl1,p<128), C=(nl0,128:176)+(nl1,128:176)
                nc.sync.dma_start(
                    y_out[0, oh * 128:(oh + 1) * 128, 0:128], outsb[:, 0:128])
                nc.sync.dma_start(
                    y_out[1, oh * 128:(oh + 1) * 128, 0:128], outsb[:, 128:256])
                nc.sync.dma_start(
                    y_out[0, oh * 128:(oh + 1) * 128, 128:176], outsb[:, 256:304])
                nc.sync.dma_start(
                    y_out[1, oh * 128:(oh + 1) * 128, 128:176], outsb[:, 304:352])

    nc.compile()
    return nc


USE_COLLECTIVE = True


def kernel(x, prototypes, temperature, w1, bn_gamma, bn_beta, w2, b2):
    x = np.ascontiguousarray(np.asarray(x, dtype=np.float32))
    prototypes = np.asarray(prototypes, dtype=np.float32)
    w1 = np.asarray(w1, dtype=np.float32)
    w2 = np.asarray(w2, dtype=np.float32)
    bn_gamma = np.asarray(bn_gamma, dtype=np.float32)
    bn_beta = np.asarray(bn_beta, dtype=np.float32)
    b2 = np.asarray(b2, dtype=np.float32)
    inv_temp = float(1.0 / np.float32(temperature))

    key = ("k", round(inv_temp, 9), USE_COLLECTIVE)
    if key not in _CACHE:
        _CACHE[key] = _build(inv_temp, USE_COLLECTIVE)
    nc = _CACHE[key]

    xr = x.reshape(N, C, S, P)
    xqf = np.ascontiguousarray(xr.transpose(0, 3, 2, 1))  # [n, P, S, C]
    pn64 = prototypes.astype(np.float64)
    pn64 = pn64 / np.maximum(np.sqrt((pn64 * pn64).sum(-1, keepdims=True)), 1e-12)
    pnc = np.ascontiguousarray(pn64.transpose(2, 0, 1).astype(np.float32))  # [C,P,K]
    w1t = np.ascontiguousarray(w1.T)
    w2t = np.ascontiguousarray(w2.T)

    def two(v):
        return np.ascontiguousarray(v.reshape(2, 128).T)

    in_maps = []
    for i in range(NCORES):
        sl = slice(i * NLOC, (i + 1) * NLOC)
        in_maps.append({
            "xc": np.ascontiguousarray(xr[sl]),
            "xq": np.ascontiguousarray(xqf[sl]),
            "pnc": pnc, "w1t": w1t, "w2t": w2t,
            "gam2": two(bn_gamma), "bet2": two(bn_beta), "b22": two(b2),
        })

    res = run_bass_kernel_spmd(nc, in_maps, core_ids=list(range(NCORES)))

    out = np.empty((N, C, P), np.float32)
    ci = np.empty((N, S), np.int32)
    for i, r in enumerate(res.results):
        out[i * NLOC:(i + 1) * NLOC] = r["y_out"]
        ci[i * NLOC:(i + 1) * NLOC] = r["ci_out"]
    return out.reshape(N, C, H, W), ci


# revision 21
# speedup vs baseline: 1.0475x; 1.0475x over previous
"""Trainium2 Bass kernel for nn_PhaseClusterAdapter.

Strategy (8 NeuronCores, data-parallel over batch n=16 -> 2 per core):
  - sim dots d[p,n,s,k] = sum_c x * proto_norm on PE (batched small matmuls,
    c-major natural layout, fp32).
  - hard assignment idx = argmax_k d (scale-invariant, so no x-normalization
    needed on this path).
  - masked max-pool ("clustered") via a fused scalar_tensor_tensor ladder in
    q=(n,p)-partition layout: acc_k = (x_s * m_k,s[q]) max acc_k.  The s=0
    step is a tensor_scalar mult, which reproduces the reference
    max_s(onehot*x) semantics exactly (zeros enter via excluded steps).
  - x row-norms via ACT Square + PE ones-matmul column sums (c-layout),
    softmax / cluster_indices path on small (n,s)-partition tiles.
  - y1 = w1 @ clustered on PE after PE-transposing clustered to kc-major.
  - BatchNorm stats: per-core partial sums + tiny AllReduce (sum, sumsq).
  - gelu fused into one ACT op (scale/bias per-partition), w2 matmul, +b2.

Host side does only sharding/layout marshalling: batch slicing, a transposed
copy of x (q-layout), prototype l2-normalization, w1/w2 transposes.
"""

import numpy as np

import concourse.bass as bass
import concourse.bacc as bacc
import concourse.tile as tile
from concourse import mybir
from concourse.bass_utils import run_bass_kernel_spmd
from concourse.masks import make_identity

F32 = mybir.dt.float32
I32 = mybir.dt.int32
AL = mybir.AluOpType
AF = mybir.ActivationFunctionType

NCORES = 8
N, C, S, H, W = 16, 256, 30, 16, 11
P = H * W            # 176
K = 4
NLOC = N // NCORES   # 2
Q = NLOC * P         # 352
NTOT = N * P         # BN count = 2816
EPS_BN = 1e-5
BIG = 1.0e6

_CACHE = {}


def _build(inv_temp: float, use_collective: bool = True):
    nc = bacc.Bacc("TRN2", target_bir_lowering=False, debug=False,
                   num_devices=NCORES)

    xc_in = nc.dram_tensor("xc", [NLOC, C, S, P], F32, kind="ExternalInput")
    xq_in = nc.dram_tensor("xq", [NLOC, P, S, C], F32, kind="ExternalInput")
    pnc_in = nc.dram_tensor("pnc", [C, P, K], F32, kind="ExternalInput")
    w1t_in = nc.dram_tensor("w1t", [K * C, C], F32, kind="ExternalInput")
    w2t_in = nc.dram_tensor("w2t", [C, C], F32, kind="ExternalInput")
    gam_in = nc.dram_tensor("gam2", [128, 2], F32, kind="ExternalInput")
    bet_in = nc.dram_tensor("bet2", [128, 2], F32, kind="ExternalInput")
    b2_in = nc.dram_tensor("b22", [128, 2], F32, kind="ExternalInput")

    y_out = nc.dram_tensor("y_out", [NLOC, C, P], F32, kind="ExternalOutput")
    ci_out = nc.dram_tensor("ci_out", [NLOC, S], I32, kind="ExternalOutput")

    # q-tiles: A = (nl0, p0:128), B = (nl1, p0:128),
    # C = (nl0, p128:176)@part0 + (nl1, p128:176)@part64
    QT = [("A", [(0, 0, 128, 0)]), ("B", [(1, 0, 128, 0)]),
          ("C", [(0, 128, 48, 0), (1, 128, 48, 64)])]
    QTD = dict(QT)

    with tile.TileContext(nc) as tc:
        import contextlib
        ctx = contextlib.ExitStack()
        with ctx:
            perm = ctx.enter_context(tc.tile_pool(name="perm", bufs=1))
            xcp = ctx.enter_context(tc.tile_pool(name="xcp", bufs=1))
            clp = ctx.enter_context(tc.tile_pool(name="clp", bufs=1))
            ps_d = ctx.enter_context(tc.tile_pool(name="ps_d", bufs=2, space="PSUM"))
            ps_t = ctx.enter_context(tc.tile_pool(name="ps_t", bufs=2, space="PSUM"))
            ps_y = ctx.enter_context(tc.tile_pool(name="ps_y", bufs=1, space="PSUM"))
            ps_o = ctx.enter_context(tc.tile_pool(name="ps_o", bufs=1, space="PSUM"))
            ps_s = ctx.enter_context(tc.tile_pool(name="ps_s", bufs=1, space="PSUM"))
            dram = ctx.enter_context(tc.tile_pool(name="dram", bufs=1, space="DRAM"))

            def dma_split(dst, srcap, nparts=128):
                """Issue a big load as two partition-halves on two queues."""
                h = nparts // 2
                nc.sync.dma_start(dst[0:h], srcap[0:h])
                nc.sync.dma_start(dst[h:nparts], srcap[h:nparts])

            # ---- small constants needed immediately ----
            pn_sb = []
            for ch in range(2):
                t = perm.tile([128, P, K], F32, tag=f"pn{ch}")
                nc.sync.dma_start(t[:], pnc_in[ch * 128:(ch + 1) * 128])
                pn_sb.append(t)
            ident = perm.tile([128, 128], F32, tag="ident")
            make_identity(nc, ident)
            iota_i = perm.tile([30, K], I32, tag="iota_i")
            nc.gpsimd.iota(iota_i[:], pattern=[[1, K]], base=0,
                           channel_multiplier=0)
            iotak = perm.tile([30, K], F32, tag="iotak")
            nc.vector.tensor_copy(iotak[:], iota_i[:])

            # ---- big loads in priority order ----
            xct = {}     # (nl, ch) -> [128, S, P]
            for ch in range(2):
                t = xcp.tile([128, S, P], F32, tag=f"xc{ch}")
                dma_split(t, xc_in[0, ch * 128:(ch + 1) * 128])
                xct[(0, ch)] = t
            xq_t = {}
            for name in ("A", "B"):
                t = clp.tile([128, S, C], F32, tag="xqbig")
                for (nl, p0, cnt, qd) in QTD[name]:
                    nc.sync.dma_start(t[qd:qd + cnt, :, :],
                                      xq_in[nl, p0:p0 + cnt])
                xq_t[name] = t

            # ---- per-nl sim dots + idx ----
            dsb, idx_f = [], []
            for nl in range(NLOC):
                if nl == 1:
                    for ch in range(2):
                        t_x = xcp.tile([128, S, P], F32, tag=f"xc{ch}")
                        dma_split(t_x, xc_in[1, ch * 128:(ch + 1) * 128])
                        xct[(1, ch)] = t_x
                d = perm.tile([30, P, K], F32, tag=f"dsb{nl}")
                for pb in range(2):
                    dp = ps_d.tile([30, 88 * K], F32, tag="dp")
                    for j in range(88):
                        p = pb * 88 + j
                        for ch in range(2):
                            nc.tensor.matmul(
                                dp[:, j * K:(j + 1) * K],
                                xct[(nl, ch)][:, :, p],
                                pn_sb[ch][:, p, :],
                                start=(ch == 0), stop=(ch == 1))
                    nc.scalar.copy(
                        d[:, pb * 88:(pb + 1) * 88, :],
                        dp[:].rearrange("s (j k) -> s j k", k=K))
                dsb.append(d)

                rmaxd = perm.tile([30, P], F32, tag=f"rmaxd{nl}")
                nc.vector.tensor_reduce(
                    rmaxd[:], d[:], axis=mybir.AxisListType.X, op=AL.max)
                pen = perm.tile([30, P, K], F32, tag="pen")
                nc.vector.tensor_tensor(
                    out=pen[:], in0=d[:],
                    in1=rmaxd[:].unsqueeze(2).broadcast_to((S, P, K)),
                    op=AL.is_lt)
                nc.vector.scalar_tensor_tensor(
                    out=pen[:], in0=pen[:], scalar=BIG,
                    in1=iotak[:].unsqueeze(1).broadcast_to((S, P, K)),
                    op0=AL.mult, op1=AL.add)
                idf = perm.tile([30, P], F32, tag=f"idf{nl}")
                nc.vector.tensor_reduce(
                    idf[:], pen[:], axis=mybir.AxisListType.X, op=AL.min)
                idx_f.append(idf)

            # ---- clT target tiles ----
            clT = {}
            for k in range(K):
                for ch in range(2):
                    clT_t = clp.tile([128, Q], F32, tag=f"clT{k}{ch}")
                    clT[(k, ch)] = clT_t
            qoff = {"A": 0, "B": 128, "C": 256}

            # ---- per q-tile: masks, ladder, transpose into clT ----
            sqj = perm.tile([128, C], F32, tag="sqj")
            rnq = {}
            for name, parts in QT:
                if name == "C":
                    t_c = clp.tile([128, S, C], F32, tag="xqbig")
                    nc.gpsimd.memset(t_c[32:64, :, :], 0.0)
                    nc.gpsimd.memset(t_c[96:128, :, :], 0.0)
                    for (nl, p0, cnt, qd) in parts:
                        nc.sync.dma_start(t_c[qd:qd + cnt, :, :],
                                          xq_in[nl, p0:p0 + cnt])
                    xq_t["C"] = t_c
                idxq = perm.tile([128, S], F32, tag=f"idxq{name}")
                if name == "C":
                    nc.vector.memset(idxq[:], 0.0)
                for (nl, p0, cnt, qd) in parts:
                    pst = ps_s.tile([128, S], F32, tag="idxt")
                    nc.tensor.transpose(pst[0:cnt, :],
                                        idx_f[nl][:, p0:p0 + cnt],
                                        ident[0:S, 0:S])
                    nc.scalar.copy(idxq[qd:qd + cnt, :], pst[0:cnt, :])
                msk = perm.tile([128, K, S], F32, tag=f"msk{name}")
                for k in range(K):
                    nc.vector.tensor_scalar(
                        out=msk[:, k, :], in0=idxq[:], scalar1=float(k),
                        scalar2=None, op0=AL.is_equal)
                xq = xq_t[name]
                for k in range(K):
                    acc = clp.tile([128, C], F32, tag=f"cl{name}{k}")
                    nc.vector.tensor_scalar(
                        out=acc[:], in0=xq[:, 0, :], scalar1=msk[:, k, 0:1],
                        scalar2=None, op0=AL.mult)
                    for s in range(1, S):
                        nc.vector.scalar_tensor_tensor(
                            out=acc[:], in0=xq[:, s, :],
                            scalar=msk[:, k, s:s + 1],
                            in1=acc[:], op0=AL.mult, op1=AL.max)
                    # transpose this tile's portion into clT right away
                    for ch in range(2):
                        col = qoff[name]
                        for (nl, p0, cnt, qd) in parts:
                            pst = ps_t.tile([128, cnt], F32, tag="pst")
                            nc.tensor.matmul(
                                pst[:],
                                acc[qd:qd + cnt, ch * 128:(ch + 1) * 128],
                                ident[qd:qd + cnt, qd:qd + cnt],
                                is_transpose=True, start=True, stop=True)
                            nc.scalar.copy(
                                clT[(k, ch)][:, col:col + cnt], pst[:])
                            col += cnt
                rq = perm.tile([128, S], F32, tag=f"rnq{name}")
                for s in range(S):
                    nc.scalar.activation(
                        sqj[:], xq[:, s, :], AF.Square,
                        accum_out=rq[:, s:s + 1])
                rnq[name] = rq

            # ---- params needed late ----
            w1sb = perm.tile([128, 8, C], F32, tag="w1sb")
            for a in range(8):
                nc.sync.dma_start(w1sb[:, a, :], w1t_in[a * 128:(a + 1) * 128, :])
            w2sb = perm.tile([128, 2, C], F32, tag="w2sb")
            for a in range(2):
                nc.sync.dma_start(w2sb[:, a, :], w2t_in[a * 128:(a + 1) * 128, :])
            gam = perm.tile([128, 2], F32, tag="gam")
            nc.sync.dma_start(gam[:], gam_in[:])
            bet = perm.tile([128, 2], F32, tag="bet")
            nc.sync.dma_start(bet[:], bet_in[:])
            b2s = perm.tile([128, 2], F32, tag="b2s")
            nc.sync.dma_start(b2s[:], b2_in[:])

            # ---- y1 = w1 @ clustered ----
            y1sb = []
            for oh in range(2):
                psy = ps_y.tile([128, Q], F32, tag="psy")
                for a in range(8):
                    k, ch = a // 2, a % 2
                    nc.tensor.matmul(
                        psy[:], w1sb[:, a, oh * 128:(oh + 1) * 128],
                        clT[(k, ch)][:], start=(a == 0), stop=(a == 7))
                t = clp.tile([128, Q], F32, tag=f"y1sb{oh}")
                nc.scalar.copy(t[:], psy[:])
                y1sb.append(t)

            # ---- BN partial stats + AllReduce ----
            stats = perm.tile([128, 4], F32, tag="stats")
            sqjunk = clp.tile([128, Q], F32, tag="sqjunk")
            for oh in range(2):
                nc.vector.tensor_reduce(
                    stats[:, 2 * oh:2 * oh + 1], y1sb[oh][:],
                    axis=mybir.AxisListType.X, op=AL.add)
                nc.vector.tensor_tensor(
                    out=sqjunk[:], in0=y1sb[oh][:], in1=y1sb[oh][:],
                    op=AL.mult)
                nc.vector.tensor_reduce(
                    stats[:, 2 * oh + 1:2 * oh + 2], sqjunk[:],
                    axis=mybir.AxisListType.X, op=AL.add)
            st_in = dram.tile([128, 4], F32)
            st_out = dram.tile([128, 4], F32)
            nc.sync.dma_start(st_in[:], stats[:])
            if use_collective:
                nc.gpsimd.collective_compute(
                    "AllReduce", AL.add, replica_groups=[list(range(NCORES))],
                    ins=[st_in.opt()], outs=[st_out.opt()])
            else:
                nc.sync.dma_start(st_out[:], st_in[:])
            stat2 = perm.tile([128, 4], F32, tag="stat2")
            nc.sync.dma_start(stat2[:], st_out[:])

            # ---- row-norm transposes ----
            rns = []
            for nl in range(NLOC):
                rn_t = perm.tile([30, P], F32, tag=f"rns{nl}")
                rns.append(rn_t)
            for name, parts in QT:
                for (nl, p0, cnt, qd) in parts:
                    pst = ps_s.tile([30, 128], F32, tag="rnt")
                    nc.tensor.transpose(pst[:, 0:cnt],
                                        rnq[name][qd:qd + cnt, :],
                                        ident[qd:qd + cnt, qd:qd + cnt])
                    nc.scalar.copy(rns[nl][:, p0:p0 + cnt], pst[:, 0:cnt])

            # ---- softmax / cluster_indices path (fills AllReduce latency) ----
            for nl in range(NLOC):
                rn, d = rns[nl], dsb[nl]
                rcp = perm.tile([30, P], F32, tag=f"rcp{nl}")
                nc.vector.reciprocal(rcp[:], rn[:])
                rs = perm.tile([30, P], F32, tag=f"rs{nl}")
                nc.scalar.activation(rs[:], rcp[:], AF.Sqrt)
                t1 = perm.tile([30, P], F32, tag=f"t1{nl}")
                nc.vector.tensor_tensor(out=t1[:], in0=rs[:], in1=rs[:], op=AL.mult)
                nc.vector.tensor_tensor(out=t1[:], in0=t1[:], in1=rn[:], op=AL.mult)
                nc.vector.tensor_scalar(out=t1[:], in0=t1[:], scalar1=-0.5,
                                        scalar2=1.5, op0=AL.mult, op1=AL.add)
                nc.vector.tensor_tensor(out=rs[:], in0=rs[:], in1=t1[:], op=AL.mult)
                sim = perm.tile([30, P, K], F32, tag="sim")
                nc.vector.tensor_tensor(
                    out=sim[:], in0=d[:],
                    in1=rs[:].unsqueeze(2).broadcast_to((S, P, K)),
                    op=AL.mult)
                smax = perm.tile([30, P], F32, tag=f"smax{nl}")
                nc.vector.tensor_reduce(
                    smax[:], sim[:], axis=mybir.AxisListType.X, op=AL.max)
                nc.vector.tensor_tensor(
                    out=sim[:], in0=sim[:],
                    in1=smax[:].unsqueeze(2).broadcast_to((S, P, K)),
                    op=AL.subtract)
                nc.scalar.activation(sim[:], sim[:], AF.Exp, scale=inv_temp)
                ssum = perm.tile([30, P], F32, tag=f"ssum{nl}")
                nc.vector.tensor_reduce(
                    ssum[:], sim[:], axis=mybir.AxisListType.X, op=AL.add)
                nc.vector.reciprocal(ssum[:], ssum[:])
                nc.vector.tensor_tensor(
                    out=sim[:], in0=sim[:],
                    in1=ssum[:].unsqueeze(2).broadcast_to((S, P, K)),
                    op=AL.mult)
                avg = perm.tile([30, K], F32, tag=f"avg{nl}")
                nc.vector.tensor_reduce(
                    avg[:], sim[:].rearrange("s p k -> s k p"),
                    axis=mybir.AxisListType.X, op=AL.add)
                amax = perm.tile([30, 1], F32, tag=f"amax{nl}")
                nc.vector.tensor_reduce(
                    amax[:], avg[:], axis=mybir.AxisListType.X, op=AL.max)
                pen2 = perm.tile([30, K], F32, tag=f"cpen{nl}")
                nc.vector.tensor_tensor(
                    out=pen2[:], in0=avg[:],
                    in1=amax[:].broadcast_to((S, K)), op=AL.is_lt)
                nc.vector.scalar_tensor_tensor(
                    out=pen2[:], in0=pen2[:], scalar=BIG, in1=iotak[:],
                    op0=AL.mult, op1=AL.add)
                cif = perm.tile([30, 1], F32, tag=f"cif{nl}")
                nc.vector.tensor_reduce(
                    cif[:], pen2[:], axis=mybir.AxisListType.X, op=AL.min)
                cii = perm.tile([30, 1], I32, tag=f"cii{nl}")
                nc.vector.tensor_copy(cii[:], cif[:])
                nc.sync.dma_start(ci_out[nl, :], cii[:, 0])

            # ---- BN scale/bias from reduced stats ----
            scale = perm.tile([128, 2], F32, tag="scale")
            shift = perm.tile([128, 2], F32, tag="shift")
            mean = perm.tile([128, 2], F32, tag="mean")
            var = perm.tile([128, 2], F32, tag="var")
            for oh in range(2):
                cnt = NTOT if use_collective else Q
                nc.vector.tensor_scalar(
                    out=mean[:, oh:oh + 1], in0=stat2[:, 2 * oh:2 * oh + 1],
                    scalar1=1.0 / cnt, scalar2=None, op0=AL.mult)
                nc.vector.tensor_scalar(
                    out=var[:, oh:oh + 1], in0=stat2[:, 2 * oh + 1:2 * oh + 2],
                    scalar1=1.0 / cnt, scalar2=None, op0=AL.mult)
            nc.vector.tensor_tensor(
                out=scale[:], in0=mean[:], in1=mean[:], op=AL.mult)
            nc.vector.tensor_tensor(
                out=var[:], in0=var[:], in1=scale[:], op=AL.subtract)
            nc.vector.tensor_scalar(
                out=var[:], in0=var[:], scalar1=EPS_BN, scalar2=None, op0=AL.add)
            rstd = perm.tile([128, 2], F32, tag="rstd")
            nc.vector.reciprocal(rstd[:], var[:])
            nc.scalar.activation(rstd[:], rstd[:], AF.Sqrt)
            t2 = perm.tile([128, 2], F32, tag="t2bn")
            nc.vector.tensor_tensor(out=t2[:], in0=rstd[:], in1=rstd[:], op=AL.mult)
            nc.vector.tensor_tensor(out=t2[:], in0=t2[:], in1=var[:], op=AL.mult)
            nc.vector.tensor_scalar(out=t2[:], in0=t2[:], scalar1=-0.5,
                                    scalar2=1.5, op0=AL.mult, op1=AL.add)
            nc.vector.tensor_tensor(out=rstd[:], in0=rstd[:], in1=t2[:], op=AL.mult)
            nc.vector.tensor_tensor(out=scale[:], in0=rstd[:], in1=gam[:], op=AL.mult)
            nc.vector.tensor_tensor(out=shift[:], in0=mean[:], in1=scale[:], op=AL.mult)
            nc.vector.tensor_tensor(out=shift[:], in0=bet[:], in1=shift[:], op=AL.subtract)

            # ---- gelu + w2 + b2 + out ----
            ysb = y1sb
            for oh in range(2):
                nc.scalar.activation(
                    y1sb[oh][:], y1sb[oh][:], AF.Gelu,
                    bias=shift[:, oh:oh + 1], scale=scale[:, oh:oh + 1])
            for oh in range(2):
                pso = ps_o.tile([128, Q], F32, tag="pso")
                for a in range(2):
                    nc.tensor.matmul(
                        pso[:], w2sb[:, a, oh * 128:(oh + 1) * 128],
                        ysb[a][:], start=(a == 0), stop=(a == 1))
                outsb = clp.tile([128, Q], F32, tag=f"outsb{oh}")
                nc.scalar.activation(
                    outsb[:], pso[:], AF.Identity, bias=b2s[:, oh:oh + 1])
                nc.sync.dma_start(
                    y_out[0, oh * 128:(oh + 1) * 128, 0:128], outsb[:, 0:128])
                nc.sync.dma_start(
                    y_out[1, oh * 128:(oh + 1) * 128, 0:128], outsb[:, 128:256])
                nc.sync.dma_start(
                    y_out[0, oh * 128:(oh + 1) * 128, 128:176], outsb[:, 256:304])
                nc.sync.dma_start(
                    y_out[1, oh * 128:(oh + 1) * 128, 128:176], outsb[:, 304:352])

    nc.compile()
    return nc


USE_COLLECTIVE = True


def kernel(x, prototypes, temperature, w1, bn_gamma, bn_beta, w2, b2):
    x = np.ascontiguousarray(np.asarray(x, dtype=np.float32))
    prototypes = np.asarray(prototypes, dtype=np.float32)
    w1 = np.asarray(w1, dtype=np.float32)
    w2 = np.asarray(w2, dtype=np.float32)
    bn_gamma = np.asarray(bn_gamma, dtype=np.float32)
    bn_beta = np.asarray(bn_beta, dtype=np.float32)
    b2 = np.asarray(b2, dtype=np.float32)
    inv_temp = float(1.0 / np.float32(temperature))

    key = ("k", round(inv_temp, 9), USE_COLLECTIVE)
    if key not in _CACHE:
        _CACHE[key] = _build(inv_temp, USE_COLLECTIVE)
    nc = _CACHE[key]

    xr = x.reshape(N, C, S, P)
    xqf = np.ascontiguousarray(xr.transpose(0, 3, 2, 1))  # [n, P, S, C]
    pn64 = prototypes.astype(np.float64)
    pn64 = pn64 / np.maximum(np.sqrt((pn64 * pn64).sum(-1, keepdims=True)), 1e-12)
    pnc = np.ascontiguousarray(pn64.transpose(2, 0, 1).astype(np.float32))  # [C,P,K]
    w1t = np.ascontiguousarray(w1.T)
    w2t = np.ascontiguousarray(w2.T)

    def two(v):
        return np.ascontiguousarray(v.reshape(2, 128).T)

    in_maps = []
    for i in range(NCORES):
        sl = slice(i * NLOC, (i + 1) * NLOC)
        in_maps.append({
            "xc": np.ascontiguousarray(xr[sl]),
            "xq": np.ascontiguousarray(xqf[sl]),
            "pnc": pnc, "w1t": w1t, "w2t": w2t,
            "gam2": two(bn_gamma), "bet2": two(bn_beta), "b22": two(b2),
        })

    res = run_bass_kernel_spmd(nc, in_maps, core_ids=list(range(NCORES)))

    out = np.empty((N, C, P), np.float32)
    ci = np.empty((N, S), np.int32)
    for i, r in enumerate(res.results):
        out[i * NLOC:(i + 1) * NLOC] = r["y_out"]
        ci[i * NLOC:(i + 1) * NLOC] = r["ci_out"]
    return out.reshape(N, C, H, W), ci
